# revision 1
# baseline (speedup 1.0000x reference)
"""Trainium2 Bass kernel for nn_ClassificationModel (CNN window encoder +
4-layer post-norm transformer + mean-pool classifier head).

Data parallel across 8 NeuronCores: batch N=64 -> 8 samples/core.
"""

import math
import sys

sys.path.insert(0, "/opt/trn_rl_repo")

import numpy as np
import ml_dtypes

import concourse.bass as bass
import concourse.mybir as mybir
import concourse.tile as tile
from concourse import bacc
from concourse.bass import AP
from concourse.bass_utils import run_bass_kernel_spmd

BF = ml_dtypes.bfloat16
F32 = mybir.dt.float32
BF16 = mybir.dt.bfloat16
AX = mybir.AxisListType
OP = mybir.AluOpType
AF = mybir.ActivationFunctionType

# model dims
N, L, W = 64, 128, 256
D, H, NL, DFF = 384, 8, 4, 1536
E = D // H  # 48
CH = [1, 4, 16, 64]
K = 7
NCORES = 8
RPC = N // NCORES          # samples per core = 8
R = RPC * L                # rows per core = 1024
TEMP = 1.0 / math.sqrt(E)
EPS = 1e-5

# conv block sizes (output positions per Toeplitz block)
B0, B1, B2 = 32, 8, 2
NB0, NB1, NB2 = 256 // B0, 128 // B1, 64 // B2  # 8, 16, 32


# ---------------------------------------------------------------------------
# host-side weight preparation
# ---------------------------------------------------------------------------

def _pe_np(l, d):
    pos = np.arange(l)[:, None].astype(np.float32)
    i = np.arange(d // 2)[None, :].astype(np.float32)
    ang = pos / np.power(10000.0, 2.0 * i / d)
    pe = np.zeros((l, d), np.float32)
    pe[:, 0::2] = np.sin(ang)
    pe[:, 1::2] = np.cos(ang)
    return pe


# conv source-block overlap enumeration (shared host/device) -----------------

# (Bout, src_size, nsrc, nch): conv0 reads xT tiles (128 pos each);
# conv1 reads pooled0 blocks (16 pos, 4 ch); conv2 reads pooled1 (4 pos, 16 ch)
CONV_GEOM = {
    0: (B0, 128, 2, 1),
    1: (B1, 16, NB0, 4),
    2: (B2, 4, NB1, 16),
}


def overlaps(conv, b):
    """source tiles overlapping output block b's input window; (src, delta)."""
    Bout, src_size, nsrc, _ = CONV_GEOM[conv]
    w0, w1 = Bout * b - 3, Bout * b + Bout + 3
    res = []
    for s in range(nsrc):
        lo, hi = s * src_size, (s + 1) * src_size
        if max(w0, lo) < min(w1, hi):
            res.append((s, lo - Bout * b))
    return res


def conv_deltas(conv):
    Bout = CONV_GEOM[conv][0]
    nb = {0: NB0, 1: NB1, 2: NB2}[conv]
    ds = sorted({d for b in range(nb) for _, d in overlaps(conv, b)})
    return ds


def _m_layout(conv, h, co):
    if conv == 0:
        return (h & 1) * 64 + (h >> 1) * 4 + co
    if conv == 1:
        return (h & 1) * 64 + (h >> 1) * 16 + co
    return h * 64 + co


def _toeplitz_variants(conv, w):
    """w: (C_out, C_in, K). returns (nvar, src_size*nch, 128) f32."""
    Bout, src_size, _, nch = CONV_GEOM[conv]
    cout = w.shape[0]
    ds = conv_deltas(conv)
    T = np.zeros((len(ds), src_size * nch, 128), np.float32)
    for vi, delta in enumerate(ds):
        for hp in range(src_size):
            for h in range(Bout):
                k = delta + hp - h + 3
                if 0 <= k < K:
                    for co in range(cout):
                        for ci in range(nch):
                            T[vi, hp * nch + ci, _m_layout(conv, h, co)] = w[co, ci, k]
    return T


def host_prep(inp):
    d = {}
    f32 = np.float32
    d["T0"] = _toeplitz_variants(0, np.asarray(inp["conv_w0"], f32)).astype(BF)
    d["T1"] = _toeplitz_variants(1, np.asarray(inp["conv_w1"], f32)).astype(BF)
    d["T2"] = _toeplitz_variants(2, np.asarray(inp["conv_w2"], f32)).astype(BF)
    b0, b1, b2 = (np.asarray(inp[f"conv_b{i}"], f32) for i in range(3))
    p = np.arange(128)
    d["b0e"] = b0[p % 4].reshape(128, 1)
    d["b1e"] = b1[p % 16].reshape(128, 1)
    d["b2e"] = b2[p % 64].reshape(128, 1)

    # embed: We_r[c, p, :] = embed_w[(p%64)*32 + 2c + p//64, :]
    ew = np.asarray(inp["embed_w"], f32)  # (2048, 384)
    We_r = np.zeros((16, 128, D), f32)
    for c in range(16):
        for pi in range(128):
            We_r[c, pi] = ew[(pi % 64) * 32 + 2 * c + pi // 64]
    d["We_r"] = We_r.astype(BF)
    d["eb_b"] = np.broadcast_to(np.asarray(inp["embed_b"], f32), (128, D)).astype(BF).copy()
    d["pe_rm"] = _pe_np(L, D)

    # Q/K outputs padded so each head gets its own 128-col chunk -> every
    # head slice sits at partition base 0 in the feature-major layout
    # (mixed PE tile_position bases crash the device).
    for nm in ("Wq", "Wk"):
        wsrc = np.asarray(inp[nm], f32)  # (4, 384, 384)
        wpad = np.zeros((NL, D, 128 * H), f32)
        for h in range(H):
            wpad[:, :, 128 * h:128 * h + 48] = wsrc[:, :, 48 * h:48 * h + 48]
        d[nm] = wpad.astype(BF)
    for nm in ("Wv", "Wo"):
        d[nm] = np.asarray(inp[nm], f32).astype(BF)  # (4, 384, 384)
    d["W1"] = np.asarray(inp["W1"], f32).astype(BF)  # (4, 384, 1536)
    d["W2"] = np.asarray(inp["W2"], f32).astype(BF)  # (4, 1536, 384)

    def _qk_bias(b):  # (4, 384) -> (4, 128, 8)
        out = np.zeros((NL, 128, 8), f32)
        for h in range(H):
            out[:, 0:48, h] = b[:, 48 * h:48 * h + 48]
        return out
    d["bq_q"] = _qk_bias(np.asarray(inp["bq"], f32))
    d["bk_q"] = _qk_bias(np.asarray(inp["bk"], f32))
    for nm, src in (("bv_b", "bv"), ("bo_b", "bo"), ("b2f_b", "b2"),
                    ("g1_b", "g1"), ("be1_b", "be1"), ("g2_b", "g2"), ("be2_b", "be2")):
        a = np.asarray(inp[src], f32)  # (4, 384)
        d[nm] = np.broadcast_to(a[:, None, :], (NL, 128, D)).astype(BF).copy()
    b1f = np.asarray(inp["b1"], f32)  # (4, 1536)
    d["b1_r"] = np.stack([b1f[l].reshape(12, 128).T for l in range(NL)])  # (4,128,12)

    d["idn_f"] = np.eye(128, dtype=f32)
    d["idn_b"] = np.eye(128, dtype=f32).astype(BF)
    d["onesL"] = np.full((128, 1), 1.0 / L, f32)
    d["clsw_r"] = np.asarray(inp["cls_w"], f32).reshape(3, 128).T.copy()  # (128,3)
    d["clsb"] = np.asarray(inp["cls_b"], f32).reshape(1, 1)
    d["epsc"] = np.full((128, 1), EPS, f32)
    return d


# ---------------------------------------------------------------------------
# device program
# ---------------------------------------------------------------------------

def build_program(debug=None, do_compile=True, n_layers=NL, phase=99):
    nc = bacc.Bacc("TRN2", target_bir_lowering=False, debug=False)

    di = {}  # dram inputs
    def dram_in(name, shape, dt=BF16):
        di[name] = nc.dram_tensor(name, list(shape), dt, kind="ExternalInput")
        return di[name]

    x_d = dram_in("xc", (R, W), F32)
    nv0, nv1, nv2 = len(conv_deltas(0)), len(conv_deltas(1)), len(conv_deltas(2))
    T0_d = dram_in("T0", (nv0, 128, 128))
    T1_d = dram_in("T1", (nv1, 64, 128))
    T2_d = dram_in("T2", (nv2, 64, 128))
    b0e_d = dram_in("b0e", (128, 1), F32)
    b1e_d = dram_in("b1e", (128, 1), F32)
    b2e_d = dram_in("b2e", (128, 1), F32)
    We_d = dram_in("We_r", (16, 128, D))
    eb_d = dram_in("eb_b", (128, D))
    pe_d = dram_in("pe_rm", (128, D), F32)
    wq_d = dram_in("Wq", (NL, D, 128 * H))
    wk_d = dram_in("Wk", (NL, D, 128 * H))
    wv_d = dram_in("Wv", (NL, D, D))
    wo_d = dram_in("Wo", (NL, D, D))
    w1_d = dram_in("W1", (NL, D, DFF))
    w2_d = dram_in("W2", (NL, DFF, D))
    bqq_d = dram_in("bq_q", (NL, 128, 8), F32)
    bkq_d = dram_in("bk_q", (NL, 128, 8), F32)
    bv_d = dram_in("bv_b", (NL, 128, D))
    bo_d = dram_in("bo_b", (NL, 128, D))
    b2f_d = dram_in("b2f_b", (NL, 128, D))
    g1_d = dram_in("g1_b", (NL, 128, D))
    be1_d = dram_in("be1_b", (NL, 128, D))
    g2_d = dram_in("g2_b", (NL, 128, D))
    be2_d = dram_in("be2_b", (NL, 128, D))
    b1r_d = dram_in("b1_r", (NL, 128, 12), F32)
    idnf_d = dram_in("idn_f", (128, 128), F32)
    idnb_d = dram_in("idn_b", (128, 128))
    onesL_d = dram_in("onesL", (128, 1), F32)
    clsw_d = dram_in("clsw_r", (128, 3), F32)
    eps_d = dram_in("epsc", (128, 1), F32)
    clsb_d = dram_in("clsb", (1, 1), F32)

    y_d = nc.dram_tensor("yc", [RPC, 1], F32, kind="ExternalOutput")
    dbg_d = None
    if debug is not None:
        dbg_d = nc.dram_tensor("dbg", [R, D], F32, kind="ExternalOutput")

    from contextlib import ExitStack
    with tile.TileContext(nc) as tc, ExitStack() as ctx:
        const = ctx.enter_context(tc.tile_pool(name="const", bufs=1))
        state = ctx.enter_context(tc.tile_pool(name="state", bufs=1))
        wpool = ctx.enter_context(tc.tile_pool(name="wpool", bufs=1))
        work = ctx.enter_context(tc.tile_pool(name="work", bufs=2))
        psum = ctx.enter_context(tc.tile_pool(name="psum", bufs=2, space="PSUM"))

        def load_const(dram, shape, dt):
            nm = dram.name + "_sb"
            t = const.tile(list(shape), dt, tag=nm, name=nm)
            nc.sync.dma_start(t[:], dram[:])
            return t

        T0v, T1v, T2v = [], [], []
        for conv, (dst, dram, npart) in enumerate(
                ((T0v, T0_d, 128), (T1v, T1_d, 64), (T2v, T2_d, 64))):
            for vi in range(len(conv_deltas(conv))):
                t = const.tile([npart, 128], BF16, tag=f"Tv{conv}_{vi}",
                               name=f"Tv{conv}_{vi}")
                nc.sync.dma_start(t[:], dram[vi])
                dst.append(t)
        d2i = [{d: i for i, d in enumerate(conv_deltas(c))} for c in range(3)]
        b0e = load_const(b0e_d, (128, 1), F32)
        b1e = load_const(b1e_d, (128, 1), F32)
        b2e = load_const(b2e_d, (128, 1), F32)
        eb_b = load_const(eb_d, (128, D), BF16)
        pe_rm = load_const(pe_d, (128, D), F32)
        idn_f = load_const(idnf_d, (128, 128), F32)
        idn_b = load_const(idnb_d, (128, 128), BF16)
        onesL = load_const(onesL_d, (128, 1), F32)
        clsw = load_const(clsw_d, (128, 3), F32)
        epsc = load_const(eps_d, (128, 1), F32)
        clsb = load_const(clsb_d, (1, 1), F32)
        We = []
        for c in range(16):
            t = const.tile([128, D], BF16, tag=f"We{c}", name=f"We{c}")
            nc.sync.dma_start(t[:], We_d[c])
            We.append(t)

        # persistent state
        t_rm = [state.tile([128, D], F32, tag=f"t_rm{rt}", name=f"t_rm{rt}") for rt in range(RPC)]
        t_fm = [state.tile([128, R], BF16, tag=f"t_fm{c}", name=f"t_fm{c}") for c in range(3)]
        o_fm = [state.tile([128, R], BF16, tag=f"o_fm{c}", name=f"o_fm{c}") for c in range(3)]
        h1 = [state.tile([128, R], BF16, tag=f"h1_{c}", name=f"h1_{c}") for c in range(12)]

        # ------------------------------------------------------- CNN + embed
        for rt in range(RPC):
            x_t = work.tile([128, W], F32, tag="x_t", name="x_t")
            nc.sync.dma_start(x_t[:], x_d[rt * 128:(rt + 1) * 128, :])

            xT = []
            for half in range(2):
                ps = psum.tile([128, 128], F32, tag="psC", name="psC")
                nc.tensor.transpose(ps[:], x_t[:, half * 128:(half + 1) * 128], idn_f[:])
                xt = work.tile([128, 128], BF16, tag=f"xT{half}", name=f"xT{half}")
                nc.scalar.copy(xt[:], ps[:])
                xT.append(xt)

            # conv0 -> pooled0 (64 = hp*4+co, 8 blocks, 128 rows)
            pooled0 = work.tile([64, NB0, 128], BF16, tag="pooled0", name="pooled0")
            for g in range(2):  # groups of 4 blocks share one psum bank
                ps = psum.tile([128, 512], F32, tag="psA", name="psA")
                for bb in range(4):
                    b = g * 4 + bb
                    ovl = overlaps(0, b)
                    for i, (s, dlt) in enumerate(ovl):
                        nc.tensor.matmul(
                            ps[:, bb * 128:(bb + 1) * 128],
                            lhsT=T0v[d2i[0][dlt]][:], rhs=xT[s][:],
                            start=(i == 0), stop=(i == len(ovl) - 1))
                r0t = work.tile([128, 512], BF16, tag="r0", name="r0")
                nc.scalar.activation(r0t[:], ps[:], AF.Relu, bias=b0e[:])
                r0s = work.tile([64, 512], BF16, tag="r0s", name="r0s")
                nc.sync.dma_start(r0s[:], r0t[64:128, :])
                nc.vector.tensor_tensor(
                    pooled0[:, g * 4:(g + 1) * 4, :],
                    r0t[0:64, :].rearrange("p (b r) -> p b r", b=4),
                    r0s[:].rearrange("p (b r) -> p b r", b=4), OP.max)

            # conv1 -> pooled1 (64 = hp*16+co, 16 blocks, 128 rows)
            pooled1 = work.tile([64, NB1, 128], BF16, tag="pooled1", name="pooled1")
            for g in range(4):
                ps = psum.tile([128, 512], F32, tag="psA", name="psA")
                for bb in range(4):
                    b = g * 4 + bb
                    ovl = overlaps(1, b)
                    for i, (s, dlt) in enumerate(ovl):
                        nc.tensor.matmul(
                            ps[:, bb * 128:(bb + 1) * 128],
                            lhsT=T1v[d2i[1][dlt]][:], rhs=pooled0[:, s, :],
                            start=(i == 0), stop=(i == len(ovl) - 1))
                r1t = work.tile([128, 512], BF16, tag="r1", name="r1")
                nc.scalar.activation(r1t[:], ps[:], AF.Relu, bias=b1e[:])
                r1s = work.tile([64, 512], BF16, tag="r1s", name="r1s")
                nc.sync.dma_start(r1s[:], r1t[64:128, :])
                nc.vector.tensor_tensor(
                    pooled1[:, g * 4:(g + 1) * 4, :],
                    r1t[0:64, :].rearrange("p (b r) -> p b r", b=4),
                    r1s[:].rearrange("p (b r) -> p b r", b=4), OP.max)

            # conv2 -> act3 (128 = (b&1)*64+co, 16 chunks, 128 rows)
            act3 = work.tile([128, 16, 128], BF16, tag="act3", name="act3")
            for g in range(8):
                ps = psum.tile([128, 512], F32, tag="psA", name="psA")
                for bb in range(4):
                    b = g * 4 + bb
                    ovl = overlaps(2, b)
                    for i, (s, dlt) in enumerate(ovl):
                        nc.tensor.matmul(
                            ps[:, bb * 128:(bb + 1) * 128],
                            lhsT=T2v[d2i[2][dlt]][:], rhs=pooled1[:, s, :],
                            start=(i == 0), stop=(i == len(ovl) - 1))
                r2t = work.tile([128, 512], BF16, tag="r2", name="r2")
                nc.scalar.activation(r2t[:], ps[:], AF.Relu, bias=b2e[:])
                r2s = work.tile([64, 512], BF16, tag="r2s", name="r2s")
                nc.sync.dma_start(r2s[:], r2t[64:128, :])
                a3t = work.tile([64, 2, 128], BF16, tag="a3t", name="a3t")
                for bb in range(4):
                    b = g * 4 + bb
                    if b & 1:
                        nc.vector.tensor_tensor(
                            a3t[:, bb >> 1, :],
                            r2t[0:64, bb * 128:(bb + 1) * 128],
                            r2s[:, bb * 128:(bb + 1) * 128], OP.max)
                    else:
                        nc.vector.tensor_tensor(
                            act3[0:64, b >> 1, :],
                            r2t[0:64, bb * 128:(bb + 1) * 128],
                            r2s[:, bb * 128:(bb + 1) * 128], OP.max)
                nc.sync.dma_start(act3[64:128, g * 2:g * 2 + 2, :], a3t[:])

            # embed (row-major out) + relu + pe
            pse = psum.tile([128, D], F32, tag="psB", name="psB")
            for c in range(16):
                nc.tensor.matmul(pse[:], lhsT=act3[:, c, :], rhs=We[c][:],
                                 start=(c == 0), stop=(c == 15))
            er = work.tile([128, D], F32, tag="er", name="er")
            nc.vector.tensor_tensor(er[:], pse[:], eb_b[:], OP.add)
            nc.scalar.activation(er[:], er[:], AF.Relu)
            nc.vector.tensor_tensor(t_rm[rt][:], er[:], pe_rm[:], OP.add)

        # ------------------------------------------------------- transformer
        for lyr in range(n_layers):
            wq = [wpool.tile([128, 128 * H], BF16, tag=f"wq{c}", name=f"wq{c}") for c in range(3)]
            wk = [wpool.tile([128, 128 * H], BF16, tag=f"wk{c}", name=f"wk{c}") for c in range(3)]
            wv = [wpool.tile([128, D], BF16, tag=f"wv{c}", name=f"wv{c}") for c in range(3)]
            wo = [wpool.tile([128, D], BF16, tag=f"wo{c}", name=f"wo{c}") for c in range(3)]
            w1 = [wpool.tile([128, DFF], BF16, tag=f"w1{c}", name=f"w1{c}") for c in range(3)]
            w2 = [wpool.tile([128, D], BF16, tag=f"w2{c}", name=f"w2{c}") for c in range(12)]
            for c in range(3):
                nc.sync.dma_start(wq[c][:], wq_d[lyr, c * 128:(c + 1) * 128, :])
                nc.sync.dma_start(wk[c][:], wk_d[lyr, c * 128:(c + 1) * 128, :])
                nc.sync.dma_start(wv[c][:], wv_d[lyr, c * 128:(c + 1) * 128, :])
                nc.sync.dma_start(wo[c][:], wo_d[lyr, c * 128:(c + 1) * 128, :])
                nc.sync.dma_start(w1[c][:], w1_d[lyr, c * 128:(c + 1) * 128, :])
            for c in range(12):
                nc.sync.dma_start(w2[c][:], w2_d[lyr, c * 128:(c + 1) * 128, :])
            bqq = wpool.tile([128, 8], F32, tag="bqq", name="bqq")
            bkq = wpool.tile([128, 8], F32, tag="bkq", name="bkq")
            nc.sync.dma_start(bqq[:], bqq_d[lyr])
            nc.sync.dma_start(bkq[:], bkq_d[lyr])
            lb = {}
            for nm, dd in (("bv", bv_d), ("bo", bo_d), ("b2f", b2f_d), ("g1", g1_d),
                           ("be1", be1_d), ("g2", g2_d), ("be2", be2_d)):
                lb[nm] = wpool.tile([128, D], BF16, tag=f"lb_{nm}", name=f"lb_{nm}")
                nc.sync.dma_start(lb[nm][:], dd[lyr])
            b1r = wpool.tile([128, 12], F32, tag="b1r", name="b1r")
            nc.sync.dma_start(b1r[:], b1r_d[lyr])

            # t_fm <- transpose(t_rm)
            for rt in range(RPC):
                for c in range(3):
                    ps = psum.tile([128, 128], F32, tag="psC", name="psC")
                    nc.tensor.transpose(ps[:], t_rm[rt][:, c * 128:(c + 1) * 128], idn_f[:])
                    nc.scalar.copy(t_fm[c][:, rt * 128:(rt + 1) * 128], ps[:])

            # attention per sample
            for n in range(RPC) if phase >= 2 else []:
                cs = slice(n * 128, (n + 1) * 128)
                # Q, K (96-part chunks), V (row-major)
                # one 128-col padded chunk per head; heads always at base 0
                qf = work.tile([64, H, 128], BF16, tag="qf", name="qf")
                kf = work.tile([64, H, 128], BF16, tag="kf", name="kf")
                for dst, wmat, bias in ((qf, wq, bqq), (kf, wk, bkq)):
                    for m in range(H):
                        pq = psum.tile([128, 128], F32, tag="psD", name="psD")
                        for c in range(3):
                            nc.tensor.matmul(pq[:], lhsT=wmat[c][:, m * 128:(m + 1) * 128],
                                             rhs=t_fm[c][:, cs], start=(c == 0), stop=(c == 2))
                        nc.scalar.activation(dst[:, m, :], pq[0:64, :], AF.Identity,
                                             bias=bias[0:64, m:m + 1])
                if phase < 3:
                    continue
                pv = psum.tile([128, D], F32, tag="psB", name="psB")
                for c in range(3):
                    nc.tensor.matmul(pv[:], lhsT=t_fm[c][:, cs], rhs=wv[c][:],
                                     start=(c == 0), stop=(c == 2))
                v_rm = work.tile([128, D], BF16, tag="v_rm", name="v_rm")
                nc.vector.tensor_tensor(v_rm[:], pv[:], lb["bv"][:], OP.add)
                if phase < 4:
                    continue

                es16 = work.tile([128, 8, 128], BF16, tag="es16", name="es16")
                nmax = work.tile([128, 8], F32, tag="nmax", name="nmax")
                ssum = work.tile([128, 8], F32, tag="ssum", name="ssum")
                rr = work.tile([128, 8], F32, tag="rr", name="rr")
                for half in range(2):
                    pss = psum.tile([128, 512], F32, tag="psA", name="psA")
                    for hh in range(4):
                        h = half * 4 + hh
                        nc.tensor.matmul(
                            pss[:, hh * 128:(hh + 1) * 128],
                            lhsT=qf[:, h, :], rhs=kf[:, h, :],
                            start=True, stop=True)
                    if phase < 5:
                        continue
                    nm4 = nmax[:, half * 4:(half + 1) * 4]
                    nc.vector.tensor_reduce(nm4, pss[:].rearrange("p (a b) -> p a b", a=4),
                                            axis=AX.X, op=OP.max, negate=True)
                    nms = work.tile([128, 4], F32, tag="nms", name="nms")
                    nc.vector.tensor_scalar(nms[:], nm4, TEMP, None, OP.mult)
                    bc = AP(nms.tensor, nms.offset, [list(nms.ap[0]), [1, 4], [0, 128]])
                    nc.vector.scalar_tensor_tensor(
                        es16[:, half * 4:(half + 1) * 4, :],
                        in0=pss[:].rearrange("p (a b) -> p a b", a=4),
                        scalar=TEMP, in1=bc, op0=OP.mult, op1=OP.add)
                    if phase < 6:
                        continue
                    nc.scalar.activation(es16[:, half * 4:(half + 1) * 4, :],
                                         es16[:, half * 4:(half + 1) * 4, :], AF.Exp)
                    nc.vector.tensor_reduce(ssum[:, half * 4:(half + 1) * 4],
                                            es16[:, half * 4:(half + 1) * 4, :],
                                            axis=AX.X, op=OP.add)
                if phase < 6:
                    continue
                nc.vector.reciprocal(rr[:], ssum[:])
                if phase < 7:
                    continue

                pso = psum.tile([128, D], F32, tag="psB", name="psB")
                for h in range(H):
                    pat = psum.tile([128, 128], BF16, tag="psC", name="psC")
                    nc.tensor.transpose(pat[:], es16[:, h, :], idn_b[:])
                    at16 = work.tile([128, 128], BF16, tag="at16", name="at16")
                    nc.vector.tensor_copy(at16[:], pat[:])
                    nc.tensor.matmul(pso[:, h * 48:(h + 1) * 48], lhsT=at16[:],
                                     rhs=v_rm[:, h * 48:(h + 1) * 48], start=True, stop=True)
                o_rm = work.tile([128, D], BF16, tag="o_rm", name="o_rm")
                rrb = AP(rr.tensor, rr.offset, [list(rr.ap[0]), [1, 8], [0, 48]])
                nc.vector.tensor_tensor(o_rm[:].rearrange("p (a b) -> p a b", a=8),
                                        pso[:].rearrange("p (a b) -> p a b", a=8),
                                        rrb, OP.mult)
                for c in range(3):
                    ps = psum.tile([128, 128], BF16, tag="psC", name="psC")
                    nc.tensor.transpose(ps[:], o_rm[:, c * 128:(c + 1) * 128], idn_b[:])
                    nc.scalar.copy(o_fm[c][:, cs], ps[:])

            # u = o @ Wo ; x1 = t + u + bo ; LN1 -> t_rm
            def layer_norm(rt, x1, gb, beb):
                bnt = work.tile([128, 6], F32, tag="bnt", name="bnt")
                ag = work.tile([128, 2], F32, tag="ag", name="ag")
                sd = work.tile([128, 1], F32, tag="sd", name="sd")
                rstd = work.tile([128, 1], F32, tag="rstd", name="rstd")
                nc.vector.bn_stats(bnt[:], x1[:])
                nc.vector.bn_aggr(ag[:], bnt[:])
                nc.scalar.activation(sd[:], ag[:, 1:2], AF.Sqrt, bias=epsc[:])
                nc.vector.reciprocal(rstd[:], sd[:])
                xn = work.tile([128, D], F32, tag="xn", name="xn")
                nc.vector.tensor_scalar(xn[:], x1[:], ag[:, 0:1], rstd[:],
                                        OP.subtract, OP.mult)
                nc.vector.tensor_tensor(xn[:], xn[:], gb[:], OP.mult)
                nc.vector.tensor_tensor(t_rm[rt][:], xn[:], beb[:], OP.add)

            for rt in range(RPC) if phase >= 8 else []:
                cs = slice(rt * 128, (rt + 1) * 128)
                pu = psum.tile([128, D], F32, tag="psB", name="psB")
                for c in range(3):
                    nc.tensor.matmul(pu[:], lhsT=o_fm[c][:, cs], rhs=wo[c][:],
                                     start=(c == 0), stop=(c == 2))
                x1 = work.tile([128, D], F32, tag="x1", name="x1")
                nc.vector.tensor_tensor(x1[:], pu[:], t_rm[rt][:], OP.add)
                nc.vector.tensor_tensor(x1[:], x1[:], lb["bo"][:], OP.add)
                layer_norm(rt, x1, lb["g1"], lb["be1"])

            # FFN
            if phase < 9:
                continue
            for rt in range(RPC):
                for c in range(3):
                    ps = psum.tile([128, 128], F32, tag="psC", name="psC")
                    nc.tensor.transpose(ps[:], t_rm[rt][:, c * 128:(c + 1) * 128], idn_f[:])
                    nc.scalar.copy(t_fm[c][:, rt * 128:(rt + 1) * 128], ps[:])
            for dc in range(12):
                for nh in range(2):
                    ph = psum.tile([128, 512], F32, tag="psA", name="psA")
                    for c in range(3):
                        nc.tensor.matmul(ph[:], lhsT=w1[c][:, dc * 128:(dc + 1) * 128],
                                         rhs=t_fm[c][:, nh * 512:(nh + 1) * 512],
                                         start=(c == 0), stop=(c == 2))
                    nc.scalar.activation(h1[dc][:, nh * 512:(nh + 1) * 512], ph[:],
                                         AF.Relu, bias=b1r[:, dc:dc + 1])
            for rt in range(RPC):
                cs = slice(rt * 128, (rt + 1) * 128)
                py = psum.tile([128, D], F32, tag="psB", name="psB")
                for dc in range(12):
                    nc.tensor.matmul(py[:], lhsT=h1[dc][:, cs], rhs=w2[dc][:],
                                     start=(dc == 0), stop=(dc == 11))
                x2 = work.tile([128, D], F32, tag="x1", name="x1")
                nc.vector.tensor_tensor(x2[:], py[:], t_rm[rt][:], OP.add)
                nc.vector.tensor_tensor(x2[:], x2[:], lb["b2f"][:], OP.add)
                layer_norm(rt, x2, lb["g2"], lb["be2"])

        if dbg_d is not None:
            for rt in range(RPC):
                nc.sync.dma_start(dbg_d[rt * 128:(rt + 1) * 128, :], t_rm[rt][:])

        # ------------------------------------------------------- head
        outsb = state.tile([1, RPC], F32, tag="outsb", name="outsb")
        for n in range(RPC):
            pm = psum.tile([128, 3], F32, tag="psC", name="psC")
            for c in range(3):
                nc.tensor.matmul(pm[:, c:c + 1], lhsT=t_rm[n][:, c * 128:(c + 1) * 128],
                                 rhs=onesL[:], start=True, stop=True)
            tm = work.tile([128, 3], F32, tag="tm", name="tm")
            nc.scalar.copy(tm[:], pm[:])
            pc = psum.tile([1, 8], F32, tag="psC", name="psC2")
            for c in range(3):
                nc.tensor.matmul(pc[:, 0:1], lhsT=tm[:, c:c + 1], rhs=clsw[:, c:c + 1],
                                 start=(c == 0), stop=(c == 2))
            nc.scalar.activation(outsb[:, n:n + 1], pc[:, 0:1], AF.Identity,
                                 bias=clsb[:])
        nc.sync.dma_start(y_d[:].rearrange("a b -> b a"), outsb[:])

    if do_compile:
        nc.compile()
    return nc


_PROG = {}


def _get_prog(debug=None, n_layers=NL, phase=99):
    key = ("dbg" if debug else "plain", n_layers, phase)
    if key not in _PROG:
        _PROG[key] = build_program(debug, n_layers=n_layers, phase=phase)
    return _PROG[key]


def _in_maps(inputs):
    shared = host_prep(inputs)
    x = np.asarray(inputs["x"], np.float32)  # (64, 128, 256)
    in_maps = []
    for c in range(NCORES):
        m = dict(shared)
        m["xc"] = np.ascontiguousarray(
            x[c * RPC:(c + 1) * RPC].reshape(R, W))
        in_maps.append(m)
    return in_maps


def kernel(**inputs):
    nc = _get_prog()
    res = run_bass_kernel_spmd(nc, _in_maps(inputs), core_ids=list(range(NCORES)))
    out = np.concatenate([res.results[c]["yc"] for c in range(NCORES)], axis=0)
    return out.astype(np.float32)


def timed_run(inputs, iters=30):
    """Wall-clock the sharded PJRT dispatch with device-resident inputs.

    No NTFF hook is available through this axon tunnel, so this measures
    dispatch+execute wall time; min over iters approximates HW exec + fixed
    dispatch overhead.  Returns ns.
    """
    import time
    import jax
    import jax.numpy as jnp
    from jax.experimental.shard_map import shard_map
    from jax.sharding import Mesh, NamedSharding, PartitionSpec
    from concourse import bass2jax, mybir as mb

    nc = _get_prog()
    bass2jax.install_neuronx_cc_hook()
    in_maps = _in_maps(inputs)
    partition_name = nc.partition_id_tensor.name if nc.partition_id_tensor else None
    in_names, out_names, out_avals, zero_outs = [], [], [], []
    for alloc in nc.m.functions[0].allocations:
        if not isinstance(alloc, mb.MemoryLocationSet):
            continue
        name = alloc.memorylocations[0].name
        if alloc.kind == "ExternalInput":
            if name != partition_name:
                in_names.append(name)
        elif alloc.kind == "ExternalOutput":
            shape = tuple(alloc.tensor_shape)
            dtype = mb.dt.np(alloc.dtype)
            out_avals.append(jax.core.ShapedArray(shape, dtype))
            out_names.append(name)
            zero_outs.append(np.zeros(shape, dtype))
    n_params, n_outs = len(in_names), len(out_avals)
    all_in = list(in_names) + list(out_names)
    if partition_name is not None:
        all_in.append(partition_name)

    def _make_body(k):
        def _body(*args):
            ins = list(args[:n_params])
            outs = list(args[n_params:])
            for _ in range(k):
                operands = ins + outs
                if partition_name is not None:
                    operands = operands + [bass2jax.partition_id_tensor()]
                outs = list(bass2jax._bass_exec_p.bind(
                    *operands, out_avals=tuple(out_avals), in_names=tuple(all_in),
                    out_names=tuple(out_names), lowering_input_output_aliases=(),
                    sim_require_finite=True, sim_require_nnan=True, nc=nc))
            return tuple(outs)
        return _body

    devices = jax.devices()[:NCORES]
    mesh = Mesh(np.asarray(devices), ("core",))
    shard = NamedSharding(mesh, PartitionSpec("core"))
    dev_in = [jax.device_put(
        np.concatenate([np.asarray(in_maps[c][nm]) for c in range(NCORES)], axis=0),
        shard) for nm in in_names]
    zsh = [np.zeros((NCORES * z.shape[0], *z.shape[1:]), z.dtype) for z in zero_outs]

    def _timeit(k, reps):
        f = jax.jit(
            shard_map(_make_body(k), mesh=mesh,
                      in_specs=(PartitionSpec("core"),) * (n_params + n_outs),
                      out_specs=(PartitionSpec("core"),) * n_outs, check_rep=False),
            keep_unused=True)
        ts = []
        for _ in range(reps):
            zs = [jax.device_put(z, shard) for z in zsh]
            jax.block_until_ready(zs)
            t0 = time.perf_counter()
            out = f(*dev_in, *zs)
            jax.block_until_ready(out)
            ts.append(time.perf_counter() - t0)
        return min(ts)

    k1, k2 = 1, 17
    _timeit(k1, 1)  # warm compile
    t1 = _timeit(k1, 5)
    t2 = _timeit(k2, 3)
    return int((t2 - t1) / (k2 - k1) * 1e9)


def debug_run(inputs, core=0, n_layers=NL, ncores=1, phase=99):
    """Run the debug program; returns (y, t_rm_dump) for one core."""
    nc = _get_prog(debug=True, n_layers=n_layers, phase=phase)
    res = run_bass_kernel_spmd(nc, _in_maps(inputs)[:ncores], core_ids=list(range(ncores)))
    return res.results[core]["yc"], res.results[core]["dbg"]



# revision 8
# speedup vs baseline: 121.0310x; 121.0310x over previous
"""Trainium2 Bass kernel for nn_ClassificationModel (CNN window encoder +
4-layer post-norm transformer + mean-pool classifier head).

Data parallel across 8 NeuronCores: batch N=64 -> 8 samples/core.
"""

import math
import sys

sys.path.insert(0, "/opt/trn_rl_repo")

import numpy as np
import ml_dtypes

import concourse.bass as bass
import concourse.mybir as mybir
import concourse.tile as tile
from concourse import bacc
from concourse.bass import AP
from concourse.bass_utils import run_bass_kernel_spmd

BF = ml_dtypes.bfloat16
F32 = mybir.dt.float32
BF16 = mybir.dt.bfloat16
AX = mybir.AxisListType
OP = mybir.AluOpType
AF = mybir.ActivationFunctionType

# model dims
N, L, W = 64, 128, 256
D, H, NL, DFF = 384, 8, 4, 1536
E = D // H  # 48
CH = [1, 4, 16, 64]
K = 7
NCORES = 8
RPC = N // NCORES          # samples per core = 8
R = RPC * L                # rows per core = 1024
TEMP = 1.0 / math.sqrt(E)
EPS = 1e-5

# conv block sizes (output positions per Toeplitz block)
B0, B1, B2 = 32, 8, 2
NB0, NB1, NB2 = 256 // B0, 128 // B1, 64 // B2  # 8, 16, 32


# ---------------------------------------------------------------------------
# host-side weight preparation
# ---------------------------------------------------------------------------

def _pe_np(l, d):
    pos = np.arange(l)[:, None].astype(np.float32)
    i = np.arange(d // 2)[None, :].astype(np.float32)
    ang = pos / np.power(10000.0, 2.0 * i / d)
    pe = np.zeros((l, d), np.float32)
    pe[:, 0::2] = np.sin(ang)
    pe[:, 1::2] = np.cos(ang)
    return pe


# conv source-block overlap enumeration (shared host/device) -----------------

# (Bout, src_size, nsrc, nch): conv0 reads xT tiles (128 pos each);
# conv1 reads pooled0 blocks (16 pos, 4 ch); conv2 reads pooled1 (4 pos, 16 ch)
CONV_GEOM = {
    0: (B0, 128, 2, 1),
    1: (B1, 16, NB0, 4),
    2: (B2, 4, NB1, 16),
}


def overlaps(conv, b):
    """source tiles overlapping output block b's input window; (src, delta)."""
    Bout, src_size, nsrc, _ = CONV_GEOM[conv]
    w0, w1 = Bout * b - 3, Bout * b + Bout + 3
    res = []
    for s in range(nsrc):
        lo, hi = s * src_size, (s + 1) * src_size
        if max(w0, lo) < min(w1, hi):
            res.append((s, lo - Bout * b))
    return res


def conv_deltas(conv):
    Bout = CONV_GEOM[conv][0]
    nb = {0: NB0, 1: NB1, 2: NB2}[conv]
    ds = sorted({d for b in range(nb) for _, d in overlaps(conv, b)})
    return ds


def _m_layout(conv, h, co):
    if conv == 0:
        return (h & 1) * 64 + (h >> 1) * 4 + co
    if conv == 1:
        return (h & 1) * 64 + (h >> 1) * 16 + co
    return h * 64 + co


def _toeplitz_variants(conv, w):
    """w: (C_out, C_in, K). returns (nvar, src_size*nch, 128) f32."""
    Bout, src_size, _, nch = CONV_GEOM[conv]
    cout = w.shape[0]
    ds = conv_deltas(conv)
    T = np.zeros((len(ds), src_size * nch, 128), np.float32)
    for vi, delta in enumerate(ds):
        for hp in range(src_size):
            for h in range(Bout):
                k = delta + hp - h + 3
                if 0 <= k < K:
                    for co in range(cout):
                        for ci in range(nch):
                            T[vi, hp * nch + ci, _m_layout(conv, h, co)] = w[co, ci, k]
    return T


def host_prep(inp):
    d = {}
    f32 = np.float32
    d["T0"] = _toeplitz_variants(0, np.asarray(inp["conv_w0"], f32)).astype(BF)
    d["T1"] = _toeplitz_variants(1, np.asarray(inp["conv_w1"], f32)).astype(BF)
    d["T2"] = _toeplitz_variants(2, np.asarray(inp["conv_w2"], f32)).astype(BF)
    b0, b1, b2 = (np.asarray(inp[f"conv_b{i}"], f32) for i in range(3))
    p = np.arange(128)
    d["b0e"] = b0[p % 4].reshape(128, 1)
    d["b1e"] = b1[p % 16].reshape(128, 1)
    d["b2e"] = b2[p % 64].reshape(128, 1)

    # embed: We_r[c, p, :] = embed_w[(p%64)*32 + 2c + p//64, :]
    ew = np.asarray(inp["embed_w"], f32)  # (2048, 384)
    We_r = np.zeros((16, 128, D), f32)
    for c in range(16):
        for pi in range(128):
            We_r[c, pi] = ew[(pi % 64) * 32 + 2 * c + pi // 64]
    d["We_r"] = We_r.astype(BF)
    d["eb_b"] = np.broadcast_to(np.asarray(inp["embed_b"], f32), (128, D)).astype(BF).copy()
    d["pe_rm"] = _pe_np(L, D)

    # Q/K outputs padded so each head gets its own 128-col chunk -> every
    # head slice sits at partition base 0 in the feature-major layout
    # (mixed PE tile_position bases crash the device).
    for nm in ("Wq", "Wk"):
        wsrc = np.asarray(inp[nm], f32)  # (4, 384, 384)
        wpad = np.zeros((NL, D, 128 * H), f32)
        for h in range(H):
            wpad[:, :, 128 * h:128 * h + 48] = wsrc[:, :, 48 * h:48 * h + 48]
        d[nm] = wpad.astype(BF)
    for nm in ("Wv", "Wo"):
        d[nm] = np.asarray(inp[nm], f32).astype(BF)  # (4, 384, 384)
    d["W1"] = np.asarray(inp["W1"], f32).astype(BF)  # (4, 384, 1536)
    d["W2"] = np.asarray(inp["W2"], f32).astype(BF)  # (4, 1536, 384)

    def _qk_bias(b):  # (4, 384) -> (4, 128, 8)
        out = np.zeros((NL, 128, 8), f32)
        for h in range(H):
            out[:, 0:48, h] = b[:, 48 * h:48 * h + 48]
        return out
    d["bq_q"] = _qk_bias(np.asarray(inp["bq"], f32))
    d["bk_q"] = _qk_bias(np.asarray(inp["bk"], f32))
    for nm, src in (("bv_b", "bv"), ("bo_b", "bo"), ("b2f_b", "b2"),
                    ("g1_b", "g1"), ("be1_b", "be1"), ("g2_b", "g2"), ("be2_b", "be2")):
        a = np.asarray(inp[src], f32)  # (4, 384)
        d[nm] = np.broadcast_to(a[:, None, :], (NL, 128, D)).astype(BF).copy()
    b1f = np.asarray(inp["b1"], f32)  # (4, 1536)
    d["b1_r"] = np.stack([b1f[l].reshape(12, 128).T for l in range(NL)])  # (4,128,12)

    d["idn_f"] = np.eye(128, dtype=f32)
    d["idn_b"] = np.eye(128, dtype=f32).astype(BF)
    d["onesL"] = np.full((128, 1), 1.0 / L, f32)
    d["clsw_r"] = np.asarray(inp["cls_w"], f32).reshape(3, 128).T.copy()  # (128,3)
    d["clsb"] = np.asarray(inp["cls_b"], f32).reshape(1, 1)
    d["epsc"] = np.full((128, 1), EPS, f32)
    return d


# ---------------------------------------------------------------------------
# device program
# ---------------------------------------------------------------------------

PHASE_MARKS = []


def _mark(nc, label):
    nm = nc.get_next_instruction_name()  # consumes one name; harmless
    PHASE_MARKS.append((label, int(nm.split("-")[-1])))


def build_program(debug=None, do_compile=True, n_layers=NL, phase=99, loop_k=1):
    PHASE_MARKS.clear()
    nc = bacc.Bacc("TRN2", target_bir_lowering=False, debug=False)

    di = {}  # dram inputs
    def dram_in(name, shape, dt=BF16):
        di[name] = nc.dram_tensor(name, list(shape), dt, kind="ExternalInput")
        return di[name]

    x_d = dram_in("xc", (R, W), F32)
    nv0, nv1, nv2 = len(conv_deltas(0)), len(conv_deltas(1)), len(conv_deltas(2))
    T0_d = dram_in("T0", (nv0, 128, 128))
    T1_d = dram_in("T1", (nv1, 64, 128))
    T2_d = dram_in("T2", (nv2, 64, 128))
    b0e_d = dram_in("b0e", (128, 1), F32)
    b1e_d = dram_in("b1e", (128, 1), F32)
    b2e_d = dram_in("b2e", (128, 1), F32)
    We_d = dram_in("We_r", (16, 128, D))
    eb_d = dram_in("eb_b", (128, D))
    pe_d = dram_in("pe_rm", (128, D), F32)
    wq_d = dram_in("Wq", (NL, D, 128 * H))
    wk_d = dram_in("Wk", (NL, D, 128 * H))
    wv_d = dram_in("Wv", (NL, D, D))
    wo_d = dram_in("Wo", (NL, D, D))
    w1_d = dram_in("W1", (NL, D, DFF))
    w2_d = dram_in("W2", (NL, DFF, D))
    bqq_d = dram_in("bq_q", (NL, 128, 8), F32)
    bkq_d = dram_in("bk_q", (NL, 128, 8), F32)
    bv_d = dram_in("bv_b", (NL, 128, D))
    bo_d = dram_in("bo_b", (NL, 128, D))
    b2f_d = dram_in("b2f_b", (NL, 128, D))
    g1_d = dram_in("g1_b", (NL, 128, D))
    be1_d = dram_in("be1_b", (NL, 128, D))
    g2_d = dram_in("g2_b", (NL, 128, D))
    be2_d = dram_in("be2_b", (NL, 128, D))
    b1r_d = dram_in("b1_r", (NL, 128, 12), F32)
    idnf_d = dram_in("idn_f", (128, 128), F32)
    idnb_d = dram_in("idn_b", (128, 128))
    onesL_d = dram_in("onesL", (128, 1), F32)
    clsw_d = dram_in("clsw_r", (128, 3), F32)
    eps_d = dram_in("epsc", (128, 1), F32)
    clsb_d = dram_in("clsb", (1, 1), F32)

    y_d = nc.dram_tensor("yc", [RPC, 1], F32, kind="ExternalOutput")
    dbg_d = None
    if debug is not None:
        dbg_d = nc.dram_tensor("dbg", [R, D], F32, kind="ExternalOutput")

    from contextlib import ExitStack
    with tile.TileContext(nc) as tc, ExitStack() as ctx:
        const = ctx.enter_context(tc.tile_pool(name="const", bufs=1))
        state = ctx.enter_context(tc.tile_pool(name="state", bufs=1))
        wpool = ctx.enter_context(tc.tile_pool(name="wpool", bufs=1))
        work = ctx.enter_context(tc.tile_pool(name="work", bufs=2))
        psum = ctx.enter_context(tc.tile_pool(name="psum", bufs=2, space="PSUM"))

        def load_const(dram, shape, dt):
            nm = dram.name + "_sb"
            t = const.tile(list(shape), dt, tag=nm, name=nm)
            nc.sync.dma_start(t[:], dram[:])
            return t

        T0v, T1v, T2v = [], [], []
        for conv, (dst, dram, npart) in enumerate(
                ((T0v, T0_d, 128), (T1v, T1_d, 64), (T2v, T2_d, 64))):
            for vi in range(len(conv_deltas(conv))):
                t = const.tile([npart, 128], BF16, tag=f"Tv{conv}_{vi}",
                               name=f"Tv{conv}_{vi}")
                nc.sync.dma_start(t[:], dram[vi])
                dst.append(t)
        d2i = [{d: i for i, d in enumerate(conv_deltas(c))} for c in range(3)]
        b0e = load_const(b0e_d, (128, 1), F32)
        b1e = load_const(b1e_d, (128, 1), F32)
        b2e = load_const(b2e_d, (128, 1), F32)
        eb_b = load_const(eb_d, (128, D), BF16)
        pe_rm = load_const(pe_d, (128, D), F32)
        idn_f = load_const(idnf_d, (128, 128), F32)
        idn_b = load_const(idnb_d, (128, 128), BF16)
        onesL = load_const(onesL_d, (128, 1), F32)
        clsw = load_const(clsw_d, (128, 3), F32)
        epsc = load_const(eps_d, (128, 1), F32)
        clsb = load_const(clsb_d, (1, 1), F32)
        We = []
        for c in range(16):
            t = const.tile([128, D], BF16, tag=f"We{c}", name=f"We{c}")
            nc.sync.dma_start(t[:], We_d[c])
            We.append(t)

        # optional hardware loop around the whole body: lets the timing
        # harness measure marginal per-iteration HW time with the fixed
        # per-exec dispatch overhead amortized away.
        if loop_k > 1:
            ctx.enter_context(tc.For_i(0, loop_k, 1))

        # persistent state
        t_rm = [state.tile([128, D], F32, tag=f"t_rm{rt}", name=f"t_rm{rt}") for rt in range(RPC)]
        t_fm = [state.tile([128, R], BF16, tag=f"t_fm{c}", name=f"t_fm{c}") for c in range(3)]
        o_fm = [state.tile([128, R], BF16, tag=f"o_fm{c}", name=f"o_fm{c}") for c in range(3)]
        h1 = [state.tile([128, R], BF16, tag=f"h1_{c}", name=f"h1_{c}") for c in range(12)]

        # ------------------------------------------------------- CNN + embed
        _mark(nc, 'cnn')
        for rt in range(RPC):
            x_t = work.tile([128, W], F32, tag="x_t", name="x_t")
            nc.sync.dma_start(x_t[:], x_d[rt * 128:(rt + 1) * 128, :])

            xT = []
            for half in range(2):
                ps = psum.tile([128, 128], F32, tag="psC", name="psC")
                nc.tensor.transpose(ps[:], x_t[:, half * 128:(half + 1) * 128], idn_f[:])
                xt = work.tile([128, 128], BF16, tag=f"xT{half}", name=f"xT{half}")
                nc.scalar.copy(xt[:], ps[:])
                xT.append(xt)

            # conv0 -> pooled0 (64 = hp*4+co, 8 blocks, 128 rows)
            pooled0 = work.tile([64, NB0, 128], BF16, tag="pooled0", name="pooled0")
            for g in range(2):  # groups of 4 blocks share one psum bank
                ps = psum.tile([128, 512], F32, tag="psA", name="psA")
                for bb in range(4):
                    b = g * 4 + bb
                    ovl = overlaps(0, b)
                    for i, (s, dlt) in enumerate(ovl):
                        nc.tensor.matmul(
                            ps[:, bb * 128:(bb + 1) * 128],
                            lhsT=T0v[d2i[0][dlt]][:], rhs=xT[s][:],
                            start=(i == 0), stop=(i == len(ovl) - 1))
                r0t = work.tile([128, 512], BF16, tag="r0", name="r0")
                nc.scalar.activation(r0t[:], ps[:], AF.Relu, bias=b0e[:])
                r0s = work.tile([64, 512], BF16, tag="r0s", name="r0s")
                nc.sync.dma_start(r0s[:], r0t[64:128, :])
                nc.vector.tensor_tensor(
                    pooled0[:, g * 4:(g + 1) * 4, :],
                    r0t[0:64, :].rearrange("p (b r) -> p b r", b=4),
                    r0s[:].rearrange("p (b r) -> p b r", b=4), OP.max)

            # conv1 -> pooled1 (64 = hp*16+co, 16 blocks, 128 rows)
            pooled1 = work.tile([64, NB1, 128], BF16, tag="pooled1", name="pooled1")
            for g in range(4):
                ps = psum.tile([128, 512], F32, tag="psA", name="psA")
                for bb in range(4):
                    b = g * 4 + bb
                    ovl = overlaps(1, b)
                    for i, (s, dlt) in enumerate(ovl):
                        nc.tensor.matmul(
                            ps[:, bb * 128:(bb + 1) * 128],
                            lhsT=T1v[d2i[1][dlt]][:], rhs=pooled0[:, s, :],
                            start=(i == 0), stop=(i == len(ovl) - 1))
                r1t = work.tile([128, 512], BF16, tag="r1", name="r1")
                nc.scalar.activation(r1t[:], ps[:], AF.Relu, bias=b1e[:])
                r1s = work.tile([64, 512], BF16, tag="r1s", name="r1s")
                nc.sync.dma_start(r1s[:], r1t[64:128, :])
                nc.vector.tensor_tensor(
                    pooled1[:, g * 4:(g + 1) * 4, :],
                    r1t[0:64, :].rearrange("p (b r) -> p b r", b=4),
                    r1s[:].rearrange("p (b r) -> p b r", b=4), OP.max)

            # conv2 -> act3 (128 = (b&1)*64+co, 16 chunks, 128 rows)
            act3 = work.tile([128, 16, 128], BF16, tag="act3", name="act3")
            for g in range(8):
                ps = psum.tile([128, 512], F32, tag="psA", name="psA")
                for bb in range(4):
                    b = g * 4 + bb
                    ovl = overlaps(2, b)
                    for i, (s, dlt) in enumerate(ovl):
                        nc.tensor.matmul(
                            ps[:, bb * 128:(bb + 1) * 128],
                            lhsT=T2v[d2i[2][dlt]][:], rhs=pooled1[:, s, :],
                            start=(i == 0), stop=(i == len(ovl) - 1))
                r2t = work.tile([128, 512], BF16, tag="r2", name="r2")
                nc.scalar.activation(r2t[:], ps[:], AF.Relu, bias=b2e[:])
                r2s = work.tile([64, 512], BF16, tag="r2s", name="r2s")
                nc.sync.dma_start(r2s[:], r2t[64:128, :])
                a3t = work.tile([64, 2, 128], BF16, tag="a3t", name="a3t")
                for bb in range(4):
                    b = g * 4 + bb
                    if b & 1:
                        nc.vector.tensor_tensor(
                            a3t[:, bb >> 1, :],
                            r2t[0:64, bb * 128:(bb + 1) * 128],
                            r2s[:, bb * 128:(bb + 1) * 128], OP.max)
                    else:
                        nc.vector.tensor_tensor(
                            act3[0:64, b >> 1, :],
                            r2t[0:64, bb * 128:(bb + 1) * 128],
                            r2s[:, bb * 128:(bb + 1) * 128], OP.max)
                nc.sync.dma_start(act3[64:128, g * 2:g * 2 + 2, :], a3t[:])

            # embed (row-major out) + relu + pe
            pse = psum.tile([128, D], F32, tag="psB", name="psB")
            for c in range(16):
                nc.tensor.matmul(pse[:], lhsT=act3[:, c, :], rhs=We[c][:],
                                 start=(c == 0), stop=(c == 15))
            er = work.tile([128, D], F32, tag="er", name="er")
            nc.vector.tensor_tensor(er[:], pse[:], eb_b[:], OP.add)
            nc.scalar.activation(er[:], er[:], AF.Relu)
            nc.vector.tensor_tensor(t_rm[rt][:], er[:], pe_rm[:], OP.add)

        # ------------------------------------------------------- transformer
        for lyr in range(n_layers):
            _mark(nc, f'L{lyr}_wload')
            wq = [wpool.tile([128, 128 * H], BF16, tag=f"wq{c}", name=f"wq{c}") for c in range(3)]
            wk = [wpool.tile([128, 128 * H], BF16, tag=f"wk{c}", name=f"wk{c}") for c in range(3)]
            wv = [wpool.tile([128, D], BF16, tag=f"wv{c}", name=f"wv{c}") for c in range(3)]
            wo = [wpool.tile([128, D], BF16, tag=f"wo{c}", name=f"wo{c}") for c in range(3)]
            w1 = [wpool.tile([128, DFF], BF16, tag=f"w1{c}", name=f"w1{c}") for c in range(3)]
            w2 = [wpool.tile([128, D], BF16, tag=f"w2{c}", name=f"w2{c}") for c in range(12)]
            for c in range(3):
                nc.sync.dma_start(wq[c][:], wq_d[lyr, c * 128:(c + 1) * 128, :])
                nc.sync.dma_start(wk[c][:], wk_d[lyr, c * 128:(c + 1) * 128, :])
                nc.sync.dma_start(wv[c][:], wv_d[lyr, c * 128:(c + 1) * 128, :])
                nc.sync.dma_start(wo[c][:], wo_d[lyr, c * 128:(c + 1) * 128, :])
                nc.sync.dma_start(w1[c][:], w1_d[lyr, c * 128:(c + 1) * 128, :])
            for c in range(12):
                nc.sync.dma_start(w2[c][:], w2_d[lyr, c * 128:(c + 1) * 128, :])
            bqq = wpool.tile([128, 8], F32, tag="bqq", name="bqq")
            bkq = wpool.tile([128, 8], F32, tag="bkq", name="bkq")
            nc.sync.dma_start(bqq[:], bqq_d[lyr])
            nc.sync.dma_start(bkq[:], bkq_d[lyr])
            lb = {}
            for nm, dd in (("bv", bv_d), ("bo", bo_d), ("b2f", b2f_d), ("g1", g1_d),
                           ("be1", be1_d), ("g2", g2_d), ("be2", be2_d)):
                lb[nm] = wpool.tile([128, D], BF16, tag=f"lb_{nm}", name=f"lb_{nm}")
                nc.sync.dma_start(lb[nm][:], dd[lyr])
            b1r = wpool.tile([128, 12], F32, tag="b1r", name="b1r")
            nc.sync.dma_start(b1r[:], b1r_d[lyr])

            # t_fm <- transpose(t_rm)
            _mark(nc, f'L{lyr}_tfm')
            for rt in range(RPC):
                for c in range(3):
                    ps = psum.tile([128, 128], F32, tag="psC", name="psC")
                    nc.tensor.transpose(ps[:], t_rm[rt][:, c * 128:(c + 1) * 128], idn_f[:])
                    nc.scalar.copy(t_fm[c][:, rt * 128:(rt + 1) * 128], ps[:])

            # attention per sample
            _mark(nc, f'L{lyr}_attn')
            for n in range(RPC) if phase >= 2 else []:
                cs = slice(n * 128, (n + 1) * 128)
                # Q, K (96-part chunks), V (row-major)
                # one 128-col padded chunk per head; heads always at base 0
                qf = work.tile([64, H, 128], BF16, tag="qf", name="qf")
                kf = work.tile([64, H, 128], BF16, tag="kf", name="kf")
                for dst, wmat, bias in ((qf, wq, bqq), (kf, wk, bkq)):
                    for m in range(H):
                        pq = psum.tile([128, 128], F32, tag="psD", name="psD")
                        for c in range(3):
                            nc.tensor.matmul(pq[:], lhsT=wmat[c][:, m * 128:(m + 1) * 128],
                                             rhs=t_fm[c][:, cs], start=(c == 0), stop=(c == 2))
                        nc.scalar.activation(dst[:, m, :], pq[0:64, :], AF.Identity,
                                             bias=bias[0:64, m:m + 1])
                if phase < 3:
                    continue
                pv = psum.tile([128, D], F32, tag="psB", name="psB")
                for c in range(3):
                    nc.tensor.matmul(pv[:], lhsT=t_fm[c][:, cs], rhs=wv[c][:],
                                     start=(c == 0), stop=(c == 2))
                v_rm = work.tile([128, D], BF16, tag="v_rm", name="v_rm")
                nc.vector.tensor_tensor(v_rm[:], pv[:], lb["bv"][:], OP.add)
                if phase < 4:
                    continue

                es16 = work.tile([128, 8, 128], BF16, tag="es16", name="es16")
                nmax = work.tile([128, 8], F32, tag="nmax", name="nmax")
                ssum = work.tile([128, 8], F32, tag="ssum", name="ssum")
                rr = work.tile([128, 8], F32, tag="rr", name="rr")
                for half in range(2):
                    pss = psum.tile([128, 512], F32, tag="psA", name="psA")
                    for hh in range(4):
                        h = half * 4 + hh
                        nc.tensor.matmul(
                            pss[:, hh * 128:(hh + 1) * 128],
                            lhsT=qf[:, h, :], rhs=kf[:, h, :],
                            start=True, stop=True)
                    if phase < 5:
                        continue
                    nm4 = nmax[:, half * 4:(half + 1) * 4]
                    nc.vector.tensor_reduce(nm4, pss[:].rearrange("p (a b) -> p a b", a=4),
                                            axis=AX.X, op=OP.max, negate=True)
                    nms = work.tile([128, 4], F32, tag="nms", name="nms")
                    nc.vector.tensor_scalar(nms[:], nm4, TEMP, None, OP.mult)
                    bc = AP(nms.tensor, nms.offset, [list(nms.ap[0]), [1, 4], [0, 128]])
                    nc.vector.scalar_tensor_tensor(
                        es16[:, half * 4:(half + 1) * 4, :],
                        in0=pss[:].rearrange("p (a b) -> p a b", a=4),
                        scalar=TEMP, in1=bc, op0=OP.mult, op1=OP.add)
                    if phase < 6:
                        continue
                    nc.scalar.activation(es16[:, half * 4:(half + 1) * 4, :],
                                         es16[:, half * 4:(half + 1) * 4, :], AF.Exp)
                    nc.vector.tensor_reduce(ssum[:, half * 4:(half + 1) * 4],
                                            es16[:, half * 4:(half + 1) * 4, :],
                                            axis=AX.X, op=OP.add)
                if phase < 6:
                    continue
                nc.vector.reciprocal(rr[:], ssum[:])
                if phase < 7:
                    continue

                pso = psum.tile([128, D], F32, tag="psB", name="psB")
                for h in range(H):
                    pat = psum.tile([128, 128], BF16, tag="psC", name="psC")
                    nc.tensor.transpose(pat[:], es16[:, h, :], idn_b[:])
                    at16 = work.tile([128, 128], BF16, tag="at16", name="at16")
                    nc.vector.tensor_copy(at16[:], pat[:])
                    nc.tensor.matmul(pso[:, h * 48:(h + 1) * 48], lhsT=at16[:],
                                     rhs=v_rm[:, h * 48:(h + 1) * 48], start=True, stop=True)
                o_rm = work.tile([128, D], BF16, tag="o_rm", name="o_rm")
                rrb = AP(rr.tensor, rr.offset, [list(rr.ap[0]), [1, 8], [0, 48]])
                nc.vector.tensor_tensor(o_rm[:].rearrange("p (a b) -> p a b", a=8),
                                        pso[:].rearrange("p (a b) -> p a b", a=8),
                                        rrb, OP.mult)
                for c in range(3):
                    ps = psum.tile([128, 128], BF16, tag="psC", name="psC")
                    nc.tensor.transpose(ps[:], o_rm[:, c * 128:(c + 1) * 128], idn_b[:])
                    nc.scalar.copy(o_fm[c][:, cs], ps[:])

            # u = o @ Wo ; x1 = t + u + bo ; LN1 -> t_rm
            _mark(nc, f'L{lyr}_wo_ln1')
            def layer_norm(rt, x1, gb, beb):
                bnt = work.tile([128, 6], F32, tag="bnt", name="bnt")
                ag = work.tile([128, 2], F32, tag="ag", name="ag")
                sd = work.tile([128, 1], F32, tag="sd", name="sd")
                rstd = work.tile([128, 1], F32, tag="rstd", name="rstd")
                nc.vector.bn_stats(bnt[:], x1[:])
                nc.vector.bn_aggr(ag[:], bnt[:])
                nc.scalar.activation(sd[:], ag[:, 1:2], AF.Sqrt, bias=epsc[:])
                nc.vector.reciprocal(rstd[:], sd[:])
                xn = work.tile([128, D], F32, tag="xn", name="xn")
                nc.vector.tensor_scalar(xn[:], x1[:], ag[:, 0:1], rstd[:],
                                        OP.subtract, OP.mult)
                nc.vector.tensor_tensor(xn[:], xn[:], gb[:], OP.mult)
                nc.vector.tensor_tensor(t_rm[rt][:], xn[:], beb[:], OP.add)

            for rt in range(RPC) if phase >= 8 else []:
                cs = slice(rt * 128, (rt + 1) * 128)
                pu = psum.tile([128, D], F32, tag="psB", name="psB")
                for c in range(3):
                    nc.tensor.matmul(pu[:], lhsT=o_fm[c][:, cs], rhs=wo[c][:],
                                     start=(c == 0), stop=(c == 2))
                x1 = work.tile([128, D], F32, tag="x1", name="x1")
                nc.vector.tensor_tensor(x1[:], pu[:], t_rm[rt][:], OP.add)
                nc.vector.tensor_tensor(x1[:], x1[:], lb["bo"][:], OP.add)
                layer_norm(rt, x1, lb["g1"], lb["be1"])

            # FFN
            _mark(nc, f'L{lyr}_ffn')
            if phase < 9:
                continue
            for rt in range(RPC):
                for c in range(3):
                    ps = psum.tile([128, 128], F32, tag="psC", name="psC")
                    nc.tensor.transpose(ps[:], t_rm[rt][:, c * 128:(c + 1) * 128], idn_f[:])
                    nc.scalar.copy(t_fm[c][:, rt * 128:(rt + 1) * 128], ps[:])
            for dc in range(12):
                for nh in range(2):
                    ph = psum.tile([128, 512], F32, tag="psA", name="psA")
                    for c in range(3):
                        nc.tensor.matmul(ph[:], lhsT=w1[c][:, dc * 128:(dc + 1) * 128],
                                         rhs=t_fm[c][:, nh * 512:(nh + 1) * 512],
                                         start=(c == 0), stop=(c == 2))
                    nc.scalar.activation(h1[dc][:, nh * 512:(nh + 1) * 512], ph[:],
                                         AF.Relu, bias=b1r[:, dc:dc + 1])
            for rt in range(RPC):
                cs = slice(rt * 128, (rt + 1) * 128)
                py = psum.tile([128, D], F32, tag="psB", name="psB")
                for dc in range(12):
                    nc.tensor.matmul(py[:], lhsT=h1[dc][:, cs], rhs=w2[dc][:],
                                     start=(dc == 0), stop=(dc == 11))
                x2 = work.tile([128, D], F32, tag="x1", name="x1")
                nc.vector.tensor_tensor(x2[:], py[:], t_rm[rt][:], OP.add)
                nc.vector.tensor_tensor(x2[:], x2[:], lb["b2f"][:], OP.add)
                layer_norm(rt, x2, lb["g2"], lb["be2"])

        if dbg_d is not None:
            for rt in range(RPC):
                nc.sync.dma_start(dbg_d[rt * 128:(rt + 1) * 128, :], t_rm[rt][:])

        # ------------------------------------------------------- head
        _mark(nc, 'head')
        outsb = state.tile([1, RPC], F32, tag="outsb", name="outsb")
        for n in range(RPC):
            pm = psum.tile([128, 3], F32, tag="psC", name="psC")
            for c in range(3):
                nc.tensor.matmul(pm[:, c:c + 1], lhsT=t_rm[n][:, c * 128:(c + 1) * 128],
                                 rhs=onesL[:], start=True, stop=True)
            tm = work.tile([128, 3], F32, tag="tm", name="tm")
            nc.scalar.copy(tm[:], pm[:])
            pc = psum.tile([1, 8], F32, tag="psC", name="psC2")
            for c in range(3):
                nc.tensor.matmul(pc[:, 0:1], lhsT=tm[:, c:c + 1], rhs=clsw[:, c:c + 1],
                                 start=(c == 0), stop=(c == 2))
            nc.scalar.activation(outsb[:, n:n + 1], pc[:, 0:1], AF.Identity,
                                 bias=clsb[:])
        nc.sync.dma_start(y_d[:].rearrange("a b -> b a"), outsb[:])

    if do_compile:
        nc.compile()
    return nc


_PROG = {}


def _get_prog(debug=None, n_layers=NL, phase=99, loop_k=1):
    key = ("dbg" if debug else "plain", n_layers, phase, loop_k)
    if key not in _PROG:
        _PROG[key] = build_program(debug, n_layers=n_layers, phase=phase,
                                   loop_k=loop_k)
    return _PROG[key]


def _in_maps(inputs):
    shared = host_prep(inputs)
    x = np.asarray(inputs["x"], np.float32)  # (64, 128, 256)
    in_maps = []
    for c in range(NCORES):
        m = dict(shared)
        m["xc"] = np.ascontiguousarray(
            x[c * RPC:(c + 1) * RPC].reshape(R, W))
        in_maps.append(m)
    return in_maps


def kernel(**inputs):
    nc = _get_prog()
    res = run_bass_kernel_spmd(nc, _in_maps(inputs), core_ids=list(range(NCORES)))
    out = np.concatenate([res.results[c]["yc"] for c in range(NCORES)], axis=0)
    return out.astype(np.float32)


def _time_exec(nc, in_maps, reps=12):
    """Min per-call wall time of one bass_exec of `nc` (device-resident IO)."""
    import time
    import jax
    from jax.experimental.shard_map import shard_map
    from jax.sharding import Mesh, NamedSharding, PartitionSpec
    from concourse import bass2jax, mybir as mb

    bass2jax.install_neuronx_cc_hook()
    partition_name = nc.partition_id_tensor.name if nc.partition_id_tensor else None
    in_names, out_names, out_avals, zero_outs = [], [], [], []
    for alloc in nc.m.functions[0].allocations:
        if not isinstance(alloc, mb.MemoryLocationSet):
            continue
        name = alloc.memorylocations[0].name
        if alloc.kind == "ExternalInput":
            if name != partition_name:
                in_names.append(name)
        elif alloc.kind == "ExternalOutput":
            shape = tuple(alloc.tensor_shape)
            dtype = mb.dt.np(alloc.dtype)
            out_avals.append(jax.core.ShapedArray(shape, dtype))
            out_names.append(name)
            zero_outs.append(np.zeros(shape, dtype))
    n_params, n_outs = len(in_names), len(out_avals)
    all_in = list(in_names) + list(out_names)
    if partition_name is not None:
        all_in.append(partition_name)

    def _body(*args):
        ins = list(args[:n_params])
        outs = list(args[n_params:])
        operands = ins + outs
        if partition_name is not None:
            operands = operands + [bass2jax.partition_id_tensor()]
        return tuple(bass2jax._bass_exec_p.bind(
            *operands, out_avals=tuple(out_avals), in_names=tuple(all_in),
            out_names=tuple(out_names), lowering_input_output_aliases=(),
            sim_require_finite=True, sim_require_nnan=True, nc=nc))

    devices = jax.devices()[:NCORES]
    mesh = Mesh(np.asarray(devices), ("core",))
    shard = NamedSharding(mesh, PartitionSpec("core"))
    dev_in = [jax.device_put(
        np.concatenate([np.asarray(in_maps[c][nm]) for c in range(NCORES)], axis=0),
        shard) for nm in in_names]
    zsh = [jax.device_put(
        np.zeros((NCORES * z.shape[0], *z.shape[1:]), z.dtype), shard)
        for z in zero_outs]
    f = jax.jit(
        shard_map(_body, mesh=mesh,
                  in_specs=(PartitionSpec("core"),) * (n_params + n_outs),
                  out_specs=(PartitionSpec("core"),) * n_outs, check_rep=False),
        keep_unused=True)
    out = f(*dev_in, *zsh)
    jax.block_until_ready(out)      # warm compile
    ts = []
    for _ in range(reps):
        t0 = time.perf_counter()
        out = f(*dev_in, *zsh)
        jax.block_until_ready(out)
        ts.append(time.perf_counter() - t0)
    host_out = [np.asarray(o) for o in out]
    return min(ts), dict(zip(out_names, host_out))


LOOP_K = 33


def timed_run(inputs, loop_k=LOOP_K, rounds=3):
    """HW exec time of one forward pass, measured as marginal cost.

    Per-exec dispatch through this axon tunnel has a large fixed overhead
    (a 1-instruction program costs the same wall time as the full kernel)
    which flips between a fast and a slow state per executable load, so
    single-shot wall time says nothing about kernel speed.  Instead we
    build the same program with the whole body wrapped in a hardware For_i
    loop of `loop_k` iterations, measure each program over several
    interleaved executable loads to find its fast-state floor, and report
        (t(loop_k) - t(1)) / (loop_k - 1)
    which cancels the fixed dispatch overhead and the one-time constant
    preamble.  Also cross-checks that the looped program computes the same
    output.  Returns ns per forward pass.
    """
    in_maps = _in_maps(inputs)
    nc1, nck = _get_prog(), _get_prog(loop_k=loop_k)
    t1s, tks = [], []
    out1 = outk = None
    for _ in range(rounds):
        t1, out1 = _time_exec(nc1, in_maps, reps=8)
        tk, outk = _time_exec(nck, in_maps, reps=8)
        t1s.append(t1)
        tks.append(tk)
    y1, yk = out1["yc"], outk["yc"]
    if not np.allclose(y1, yk, atol=1e-5, rtol=1e-3):
        print(f"WARNING: loop_k output mismatch (max abs diff "
              f"{np.abs(y1 - yk).max():.3e})")
    return int((min(tks) - min(t1s)) / (loop_k - 1) * 1e9)


def debug_run(inputs, core=0, n_layers=NL, ncores=1, phase=99):
    """Run the debug program; returns (y, t_rm_dump) for one core."""
    nc = _get_prog(debug=True, n_layers=n_layers, phase=phase)
    res = run_bass_kernel_spmd(nc, _in_maps(inputs)[:ncores], core_ids=list(range(ncores)))
    return res.results[core]["yc"], res.results[core]["dbg"]

